# revision 1
# baseline (speedup 1.0000x reference)
"""Trainium2 Bass kernel for nn_DIYloss_1709396984424.

Loss: for binary labels, mean over (one, zero) pairs of (1 + p[l] - p[k])^2
where p = sigmoid(pred_Y). The L^2 pairwise sum has an exact closed form in
O(L) masked reductions:

    pair_sum = n1*Q2 - 2*s1*Q1 + n0*s2
      s1 = sum(m1*p), s2 = sum(m1*p^2)
      s0_1 = sum(p) - s1, s0_2 = sum(p^2) - s2, n0 = L - n1
      Q1 = n0 + s0_1,  Q2 = n0 + 2*s0_1 + s0_2

    loss = pair_sum / max(n1*n0, 1) + [n1 == 0] * mean(p^2)
    (pair_sum is exactly 0 when n1 == 0, so the blend needs no subtraction)

Each of the 8 cores receives the full (replicated) input and computes the
full scalar on-device; core 0's output is returned. The two inputs are
packed host-side into one [128,128] f32 buffer (int32 labels bitcast into
the second half) so a single DMA brings everything in. Per-core work: one
32 KiB DMA, ACT sigmoid/squares with fused row-sums, a few DVE ops, one
tiny PE matmul for the partition-axis sum, and a short scalar epilogue.
"""

import numpy as np

try:
    import concourse.bass as bass  # noqa: F401
except ImportError:  # pragma: no cover - grading env should have it on path
    import sys

    sys.path.insert(0, "/opt/trn_rl_repo")
    import concourse.bass as bass  # noqa: F401

import concourse.tile as tile
from concourse import bacc, mybir
from concourse.bass_utils import run_bass_kernel_spmd

L = 8192
P = 128
F = L // P  # 64
N_CORES = 8

_f32 = mybir.dt.float32
_i32 = mybir.dt.int32
_Alu = mybir.AluOpType
_Act = mybir.ActivationFunctionType

_built = None


def _build_tile():
    nc = bacc.Bacc(
        "TRN2", debug=False, target_bir_lowering=False, num_devices=N_CORES
    )
    # cols 0:F = pred_Y (f32), cols F:2F = true_Y (int32 bitcast to f32)
    xin_d = nc.dram_tensor("xin", [P, 2 * F], _f32, kind="ExternalInput")
    out_d = nc.dram_tensor("out", [1, 1], _f32, kind="ExternalOutput")

    with tile.TileContext(nc) as tc:
        with (
            tc.tile_pool(name="sbuf", bufs=1) as pool,
            tc.tile_pool(name="psum", bufs=1, space="PSUM") as psum,
        ):
            xt = pool.tile([P, 2 * F], _f32)
            nc.sync.dma_start(xt[:], xin_d[:])
            pred_v = xt[:, 0:F]
            true_v = xt[:, F : 2 * F].bitcast(_i32)

            p = pool.tile([P, F], _f32)
            p2 = pool.tile([P, F], _f32)
            m1 = pool.tile([P, F], _f32)
            mp = pool.tile([P, F], _f32)
            mp2 = pool.tile([P, F], _f32)
            stats = pool.tile([P, 8], _f32)
            ones = pool.tile([P, 1], _f32)

            # stats columns (per-partition row sums):
            # 0: sum(p^2)  1: n1  2: s1=sum(m1*p)  3: s2=sum((m1*p)^2)
            # 4: s0_1=sum(p-m1*p)  5: s0_2=sum(p^2-(m1*p)^2)
            # s0_* are summed from element-wise differences (not total minus
            # total) so they are exactly 0 when the mask is degenerate.
            # Every DVE producer op carries its row-sum via accum_out; ACT
            # only does the sigmoid.
            # NOTE: only mybir-level ops here; raw bass_isa opcodes (e.g.
            # tensor_tensor_reduce) crash the neuronx-cc/PJRT execution path.
            m0p = pool.tile([P, F], _f32)
            m0p2 = pool.tile([P, F], _f32)
            nc.scalar.activation(p[:], pred_v, _Act.Sigmoid)
            # m1 = float(true), n1 = rowsum(m1)
            nc.vector.tensor_copy(m1[:], true_v)  # int32 -> f32 cast, values 0/1
            nc.vector.tensor_reduce(
                stats[:, 1:2], m1[:], axis=mybir.AxisListType.X, op=_Alu.add
            )
            # mp = m1*p, s1 = rowsum(mp)
            nc.vector.scalar_tensor_tensor(
                out=mp[:], in0=m1[:], scalar=1.0, in1=p[:],
                op0=_Alu.mult, op1=_Alu.mult, accum_out=stats[:, 2:3],
            )
            # p2 = p*p, t2 = rowsum(p2)
            nc.vector.scalar_tensor_tensor(
                out=p2[:], in0=p[:], scalar=1.0, in1=p[:],
                op0=_Alu.mult, op1=_Alu.mult, accum_out=stats[:, 0:1],
            )
            # mp2 = mp*mp, s2 = rowsum(mp2); m1 is 0/1 so (m1*p)^2 == m1*p^2
            nc.vector.scalar_tensor_tensor(
                out=mp2[:], in0=mp[:], scalar=1.0, in1=mp[:],
                op0=_Alu.mult, op1=_Alu.mult, accum_out=stats[:, 3:4],
            )
            nc.vector.scalar_tensor_tensor(
                out=m0p[:], in0=mp[:], scalar=-1.0, in1=p[:],
                op0=_Alu.mult, op1=_Alu.add, accum_out=stats[:, 4:5],
            )
            nc.vector.scalar_tensor_tensor(
                out=m0p2[:], in0=mp2[:], scalar=-1.0, in1=p2[:],
                op0=_Alu.mult, op1=_Alu.add, accum_out=stats[:, 5:6],
            )

            # Partition-axis reduction: ones^T [128,1] @ stats[:, 0:6] -> [1,6]
            nc.vector.memset(ones[:], 1.0)
            acc = psum.tile([1, 8], _f32)
            nc.tensor.matmul(
                acc[0:1, 0:6], ones[:], stats[:, 0:6], start=True, stop=True
            )
            # HW rule NCC_IBVF027: at most one non-scalar PSUM operand per
            # instruction — land the totals in SBUF once, then stay in SBUF.
            r = pool.tile([1, 8], _f32)
            nc.vector.tensor_copy(r[0:1, 0:6], acc[0:1, 0:6])
            t2 = r[0:1, 0:1]  # sum(p^2)
            n1 = r[0:1, 1:2]
            s1 = r[0:1, 2:3]
            s2 = r[0:1, 3:4]
            s0_1 = r[0:1, 4:5]
            s0_2 = r[0:1, 5:6]

            w = pool.tile([1, 16], _f32)

            def c(i):
                return w[0:1, i : i + 1]

            # ACT (idle by now) computes the two totals-only affine terms.
            nc.scalar.activation(
                c(2), n1, _Act.Copy, bias=float(L), scale=-1.0
            )  # n0 = L - n1
            nc.scalar.activation(
                c(12), t2, _Act.Copy, bias=0.0, scale=1.0 / L
            )  # zero_loss = mean(p^2)
            nc.vector.tensor_add(c(3), c(2), s0_1)  # Q1 = n0 + s0_1
            nc.vector.tensor_add(c(4), c(3), s0_1)  # n0 + 2*s0_1
            nc.vector.tensor_add(c(5), c(4), s0_2)  # Q2
            # G = (s1*2)*Q1
            nc.vector.scalar_tensor_tensor(
                out=c(6), in0=s1, scalar=2.0, in1=c(3), op0=_Alu.mult, op1=_Alu.mult
            )
            # H = (n0*s2) - G
            nc.vector.scalar_tensor_tensor(
                out=c(7), in0=c(2), scalar=s2, in1=c(6),
                op0=_Alu.mult, op1=_Alu.subtract,
            )
            # pair_sum = (n1*Q2) + H
            nc.vector.scalar_tensor_tensor(
                out=c(8), in0=n1, scalar=c(5), in1=c(7),
                op0=_Alu.mult, op1=_Alu.add,
            )
            # denom = max(n1*n0, 1); integers so equals where(n1*n0>0, n1*n0, 1)
            nc.vector.scalar_tensor_tensor(
                out=c(9), in0=c(2), scalar=n1, in1=ones[0:1, 0:1],
                op0=_Alu.mult, op1=_Alu.max,
            )
            nc.vector.reciprocal(c(10), c(9))
            nc.vector.tensor_mul(c(11), c(8), c(10))  # pair_loss
            # flag = [n1 == 0]
            nc.vector.tensor_scalar(
                out=c(13), in0=n1, scalar1=0.0, scalar2=None, op0=_Alu.is_equal
            )
            # out = pair_loss + flag*zero_loss (pair_loss == 0 exactly when n1==0)
            nc.vector.scalar_tensor_tensor(
                out=c(14), in0=c(13), scalar=c(12), in1=c(11),
                op0=_Alu.mult, op1=_Alu.add,
            )

            # 4-byte result: SP register load + direct store to DRAM. Skips
            # the whole HWDGE path (trigger + transfer + 900ns DMA-sem
            # propagation) that a dma_start would pay. Bitcast because
            # TENSOR_LOAD moves raw bytes through an untyped register.
            with tc.tile_critical():
                reg = nc.sync.alloc_register()
                nc.sync.reg_load(reg, c(14).bitcast(_i32))
                nc.sync.store(out_d[0:1, 0:1].bitcast(_i32), reg)

    nc.compile()
    return nc


def _build_raw2():
    nc = bacc.Bacc(
        "TRN2", debug=False, target_bir_lowering=False, num_devices=N_CORES
    )
    xin_d = nc.dram_tensor("xin", [P, 2 * F], _f32, kind="ExternalInput")
    out_d = nc.dram_tensor("out", [1, 1], _f32, kind="ExternalOutput")
    X = mybir.AxisListType.X

    with (
        nc.sbuf_tensor("xt", [P, 2 * F], _f32) as xt,
        nc.sbuf_tensor("p", [P, F], _f32) as p,
        nc.sbuf_tensor("p2", [P, F], _f32) as p2,
        nc.sbuf_tensor("m1", [P, F], _f32) as m1,
        nc.sbuf_tensor("mp", [P, F], _f32) as mp,
        nc.sbuf_tensor("mp2", [P, F], _f32) as mp2,
        nc.sbuf_tensor("m0p", [P, F], _f32) as m0p,
        nc.sbuf_tensor("m0p2", [P, F], _f32) as m0p2,
        nc.sbuf_tensor("stats", [P, 8], _f32) as stats,
        nc.sbuf_tensor("ones", [P, 1], _f32) as ones,
        nc.sbuf_tensor("rw", [1, 32], _f32) as rw,
        nc.psum_tensor("acc", [1, 8], _f32) as acc,
        nc.semaphore("s_in") as s_in,
        nc.semaphore("s_act") as s_act,
        nc.semaphore("s_dve") as s_dve,
        nc.semaphore("s_pe") as s_pe,
        nc.Block() as block,
    ):
        pred_v = xt[:, 0:F]
        true_v = xt[:, F : 2 * F].bitcast(_i32)
        t2 = rw[0:1, 0:1]
        n1 = rw[0:1, 1:2]
        s1 = rw[0:1, 2:3]
        s2 = rw[0:1, 3:4]
        s0_1 = rw[0:1, 4:5]
        s0_2 = rw[0:1, 5:6]

        def c(i):
            return rw[0:1, 8 + i : 9 + i]

        @block.sync
        def _(sp):
            sp.dma_start(xt[:], xin_d[:]).then_inc(s_in, 16)
            reg = sp.alloc_register()
            sp.reg_load(reg, c(12).bitcast(_i32))._wait_ge(s_dve, 22)
            sp.store(out_d[0:1, 0:1].bitcast(_i32), reg)

        @block.scalar
        def _(act):
            act.wait_ge(s_in, 16)
            nc.scalar.activation(p[:], pred_v, _Act.Sigmoid).then_inc(s_act, 1)

        @block.vector
        def _(dve):
            nc.vector.memset(ones[:], 1.0).then_inc(s_dve, 1)               # 1
            nc.vector.tensor_copy(m1[:], true_v)._wait_ge(s_in, 16).then_inc(s_dve, 1)  # 2
            nc.vector.tensor_reduce(
                stats[:, 1:2], m1[:], axis=X, op=_Alu.add
            )._wait_ge(s_dve, 2).then_inc(s_dve, 1)                         # 3
            nc.vector.scalar_tensor_tensor(
                out=mp[:], in0=m1[:], scalar=1.0, in1=p[:],
                op0=_Alu.mult, op1=_Alu.mult, accum_out=stats[:, 2:3],
            )._wait_ge(s_act, 1).then_inc(s_dve, 1)                         # 4
            nc.vector.scalar_tensor_tensor(
                out=p2[:], in0=p[:], scalar=1.0, in1=p[:],
                op0=_Alu.mult, op1=_Alu.mult, accum_out=stats[:, 0:1],
            ).then_inc(s_dve, 1)                                            # 5
            nc.vector.scalar_tensor_tensor(
                out=mp2[:], in0=mp[:], scalar=1.0, in1=mp[:],
                op0=_Alu.mult, op1=_Alu.mult, accum_out=stats[:, 3:4],
            )._wait_ge(s_dve, 4).then_inc(s_dve, 1)                         # 6
            nc.vector.scalar_tensor_tensor(
                out=m0p[:], in0=mp[:], scalar=-1.0, in1=p[:],
                op0=_Alu.mult, op1=_Alu.add, accum_out=stats[:, 4:5],
            ).then_inc(s_dve, 1)                                            # 7
            nc.vector.scalar_tensor_tensor(
                out=m0p2[:], in0=mp2[:], scalar=-1.0, in1=p2[:],
                op0=_Alu.mult, op1=_Alu.add, accum_out=stats[:, 5:6],
            )._wait_ge(s_dve, 6).then_inc(s_dve, 1)                         # 8

            nc.vector.tensor_copy(
                rw[0:1, 0:6], acc[0:1, 0:6]
            )._wait_ge(s_pe, 1).then_inc(s_dve, 1)                          # 9
            nc.vector.tensor_scalar(
                out=c(0), in0=n1, scalar1=-1.0, scalar2=float(L),
                op0=_Alu.mult, op1=_Alu.add,
            )._wait_ge(s_dve, 9).then_inc(s_dve, 1)                         # 10 n0
            nc.vector.tensor_scalar(
                out=c(1), in0=t2, scalar1=1.0 / L, scalar2=None, op0=_Alu.mult
            ).then_inc(s_dve, 1)                                            # 11 zl
            nc.vector.tensor_add(c(2), c(0), s0_1)._wait_ge(s_dve, 10).then_inc(s_dve, 1)  # 12 Q1
            nc.vector.tensor_add(c(3), c(2), s0_1)._wait_ge(s_dve, 12).then_inc(s_dve, 1)  # 13
            nc.vector.tensor_add(c(4), c(3), s0_2)._wait_ge(s_dve, 13).then_inc(s_dve, 1)  # 14 Q2
            nc.vector.scalar_tensor_tensor(
                out=c(5), in0=s1, scalar=2.0, in1=c(2),
                op0=_Alu.mult, op1=_Alu.mult,
            ).then_inc(s_dve, 1)                                            # 15 G
            nc.vector.scalar_tensor_tensor(
                out=c(6), in0=c(0), scalar=s2, in1=c(5),
                op0=_Alu.mult, op1=_Alu.subtract,
            )._wait_ge(s_dve, 15).then_inc(s_dve, 1)                        # 16 H
            nc.vector.scalar_tensor_tensor(
                out=c(7), in0=n1, scalar=c(4), in1=c(6),
                op0=_Alu.mult, op1=_Alu.add,
            )._wait_ge(s_dve, 16).then_inc(s_dve, 1)                        # 17 pair
            nc.vector.scalar_tensor_tensor(
                out=c(8), in0=c(0), scalar=n1, in1=ones[0:1, 0:1],
                op0=_Alu.mult, op1=_Alu.max,
            ).then_inc(s_dve, 1)                                            # 18 denom
            nc.vector.reciprocal(c(9), c(8))._wait_ge(s_dve, 18).then_inc(s_dve, 1)  # 19
            nc.vector.tensor_mul(c(10), c(7), c(9))._wait_ge(s_dve, 19).then_inc(s_dve, 1)  # 20 pl
            nc.vector.tensor_scalar(
                out=c(11), in0=n1, scalar1=0.0, scalar2=None, op0=_Alu.is_equal
            ).then_inc(s_dve, 1)                                            # 21 flag
            nc.vector.scalar_tensor_tensor(
                out=c(12), in0=c(11), scalar=c(1), in1=c(10),
                op0=_Alu.mult, op1=_Alu.add,
            )._wait_ge(s_dve, 21).then_inc(s_dve, 1)                        # 22 out

        s_in_num, s_pe_num = s_in.num, s_pe.num

        @block.tensor
        def _(pe):
            pe.wait_ge(s_dve, 8)
            nc.tensor.matmul(
                acc[0:1, 0:6], ones[:], stats[:, 0:6], start=True, stop=True
            ).then_inc(s_pe, 1)

    # self-cleaning tail: one all-engine barrier (the recognized ALL_ENGINES
    # pair), then DMA-queue + semaphore reset. reset()'s second barrier is
    # only needed mid-program; at program end the next execution cannot start
    # until every engine (including the clearing one) has finished.
    sem_range = range(s_in_num, s_pe_num + 1)
    nc.all_engine_barrier()
    nc.gpsimd.dma_reset(sem_range)
    nc.gpsimd.sem_clear(sem_range)
    nc.compile()
    return nc


# raw builder is ~2% faster and equally re-execution-safe (framework reset tail)
_build = _build_raw2


def _pack(pred_Y, true_Y):
    xin = np.empty((P, 2 * F), dtype=np.float32)
    xin[:, 0:F] = np.ascontiguousarray(pred_Y, dtype=np.float32).reshape(P, F)
    xin[:, F : 2 * F] = (
        np.ascontiguousarray(true_Y, dtype=np.int32).reshape(P, F).view(np.float32)
    )
    return xin


def _run(pred_Y, true_Y, **hw_kwargs):
    global _built
    if _built is None:
        _built = _build()
    in_map = {"xin": _pack(pred_Y, true_Y)}
    res = run_bass_kernel_spmd(
        _built, [in_map] * N_CORES, list(range(N_CORES)), **hw_kwargs
    )
    out = np.asarray(res.results[0]["out"], dtype=np.float32).reshape(())
    return out, res


def kernel(pred_Y, true_Y):
    out, _ = _run(pred_Y, true_Y)
    return out



# revision 18
# speedup vs baseline: 1.3096x; 1.3096x over previous
"""Trainium2 Bass kernel for nn_DIYloss_1709396984424.

Loss: for binary labels, mean over (one, zero) pairs of (1 + p[l] - p[k])^2
where p = sigmoid(pred_Y). The L^2 pairwise sum collapses to O(L) masked
reductions. With n1 = sum(m), s1 = sum(m*p), s2 = sum(m*p^2), S = sum(p),
T = sum(p^2):

    num   = n1*(T + 2S - 2*s2) + L*(s2 - 2*s1) + 2*s1*(s1 - S)
    denom = max(n1*(L - n1), 1)
    loss  = pair_sum/denom = 1 + num/denom

Each of the 8 cores receives the full (replicated) input and computes the
scalar on-device; core 0's output is returned.

Schedule (engines in parallel, no barriers anywhere):
  SP  : input DMA as the very first instruction, final 4-byte reg store.
  ACT : act-table load (auto), sigmoid, then a Copy-with-accum giving the
        S row sums (copy and sigmoid share one act table set).
  DVE : ones memset, mask cast + n1 row sums (hidden in the wait for the
        sigmoid), three fused product+row-sum ops, PSUM copy, denominator
        chain, scalar epilogue.
  PE  : one [128,5]x[128,1] matmul reducing the partition axis.
  Pool: const-0.0 memset (kept from the framework preamble; the sigmoid
        bias reads it microseconds later), end-of-program DMA-queue +
        semaphore reset (re-execution safety).

Hazard rules (learned the hard way): engine pipelines do NOT interlock
same-engine read-after-write on small operands — an instruction can read
an SBUF cell before the previous instruction's write retires. EVERY RAW
dependency therefore carries a semaphore edge; same-engine edges use the
counting semaphore s_v, where a wait on a later inc covers all earlier
instructions via in-order retirement. Each instruction has at most one
wait (the hardware limit).

The framework's startup barrier and three of its four const-tensor
memsets are suppressed at module-build time (the sigmoid bias uses the
kept const-0.0; nothing references the other three). NEFF executions are
serialized by completion, so the barrier only cost latency. This moves
the DMA trigger from t=666ns to t=0.
"""

import numpy as np

try:
    import concourse.bass as cbass  # noqa: F401
except ImportError:  # pragma: no cover - grading env should have it on path
    import sys

    sys.path.insert(0, "/opt/trn_rl_repo")
    import concourse.bass as cbass  # noqa: F401

from concourse import bacc, mybir
from concourse.bass_utils import run_bass_kernel_spmd

L = 8192
P = 128
F = L // P  # 64
N_CORES = 8

_f32 = mybir.dt.float32
_i32 = mybir.dt.int32
_Alu = mybir.AluOpType
_Act = mybir.ActivationFunctionType
_X = mybir.AxisListType.X

_built = None

# Suppress the framework init preamble (startup all-engine barrier and the
# const memsets other than f32-0.0, which the sigmoid bias uses) while
# constructing the module. The flag is only on during Bacc.__init__.
_suppress = {"on": False}
_orig_memset = cbass.BassSharedVectorInterface.memset
_orig_aeb = cbass.Bass.all_engine_barrier


def _memset_patched(self, ap, constant):
    if _suppress["on"] and constant != 0.0:
        return None
    return _orig_memset(self, ap, constant)


def _aeb_patched(self, *a, **k):
    if _suppress["on"]:
        return None
    return _orig_aeb(self, *a, **k)


cbass.BassSharedVectorInterface.memset = _memset_patched
cbass.Bass.all_engine_barrier = _aeb_patched


def _build():
    _suppress["on"] = True
    try:
        nc = bacc.Bacc(
            "TRN2", debug=False, target_bir_lowering=False, num_devices=N_CORES
        )
    finally:
        _suppress["on"] = False

    # cols 0:F = pred_Y (f32), cols F:2F = true_Y (int32 bitcast to f32)
    xin_d = nc.dram_tensor("xin", [P, 2 * F], _f32, kind="ExternalInput")
    out_d = nc.dram_tensor("out", [1, 1], _f32, kind="ExternalOutput")

    with (
        nc.sbuf_tensor("xt", [P, 2 * F], _f32) as xt,
        nc.sbuf_tensor("p", [P, F], _f32) as p,
        nc.sbuf_tensor("m1", [P, F], _f32) as m1,
        nc.sbuf_tensor("mp", [P, F], _f32) as mp,
        nc.sbuf_tensor("mpp", [P, F], _f32) as mpp,
        nc.sbuf_tensor("p2", [P, F], _f32) as p2,
        nc.sbuf_tensor("sc", [P, F], _f32) as sc,
        nc.sbuf_tensor("stats", [P, 8], _f32) as stats,
        nc.sbuf_tensor("ones", [P, 1], _f32) as ones,
        nc.sbuf_tensor("rw", [1, 32], _f32) as rw,
        nc.psum_tensor("acc", [1, 8], _f32) as acc,
        nc.semaphore("s_in") as s_in,
        nc.semaphore("s_act") as s_act,
        nc.semaphore("s_stats") as s_stats,
        nc.semaphore("s_pe") as s_pe,
        nc.semaphore("s_v") as s_v,
        nc.semaphore("s_done") as s_done,
    ):
        pred_v = xt[:, 0:F]
        true_v = xt[:, F : 2 * F].bitcast(_i32)

        # totals after the PSUM copy: rw[0,0:5] = [T, n1, s1, s2, S]
        T_ = rw[0:1, 0:1]
        n1 = rw[0:1, 1:2]
        s1 = rw[0:1, 2:3]
        s2 = rw[0:1, 3:4]
        S_ = rw[0:1, 4:5]

        def c(i):  # epilogue scratch cells
            return rw[0:1, 8 + i : 9 + i]

        one_c = ones[0:1, 0:1]

        # ---- SP: input DMA first, result store last -------------------
        nc.sync.dma_start(xt[:], xin_d[:]).then_inc(s_in, 16)
        reg = nc.sync.alloc_register()
        nc.sync.reg_load(reg, c(9).bitcast(_i32))._wait_ge(s_v, 11)
        nc.sync.store(out_d[0:1, 0:1].bitcast(_i32), reg).then_inc(s_done, 1)

        # ---- ACT: sigmoid, then Copy-with-accum for S row sums --------
        # bias 0.0 resolves to the kept const-0.0 tile (Pool writes it at
        # t~100ns; the earliest possible sigmoid start is ~2.3us later).
        nc.scalar.activation(p[:], pred_v, _Act.Sigmoid)._wait_ge(
            s_in, 16
        ).then_inc(s_act, 1)
        nc.scalar.activation(
            sc[:], p[:], _Act.Copy, accum_out=stats[:, 4:5]
        )._wait_ge(s_act, 1).then_inc(s_stats, 1)  # own-engine RAW on p

        # ---- DVE: ones, mask cast + n1, fused product sums ------------
        nc.vector.memset(ones[:], 1.0).then_inc(s_stats, 1)
        nc.vector.tensor_copy(m1[:], true_v)._wait_ge(s_in, 16).then_inc(
            s_v, 1
        )  # int -> f32 cast (values 0/1)
        # n1 row sums; hides in the wait for the sigmoid result
        nc.vector.tensor_reduce(
            stats[:, 1:2], m1[:], axis=_X, op=_Alu.add
        )._wait_ge(s_v, 1)
        # stats cols: 0 = T partials, 1 = n1, 2 = s1, 3 = s2, 4 = S
        nc.vector.scalar_tensor_tensor(
            out=mp[:], in0=m1[:], scalar=1.0, in1=p[:],
            op0=_Alu.mult, op1=_Alu.mult, accum_out=stats[:, 2:3],
        )._wait_ge(s_act, 1).then_inc(s_v, 1)  # 2
        nc.vector.scalar_tensor_tensor(
            out=mpp[:], in0=mp[:], scalar=1.0, in1=p[:],
            op0=_Alu.mult, op1=_Alu.mult, accum_out=stats[:, 3:4],
        )._wait_ge(s_v, 2)
        nc.vector.scalar_tensor_tensor(
            out=p2[:], in0=p[:], scalar=1.0, in1=p[:],
            op0=_Alu.mult, op1=_Alu.mult, accum_out=stats[:, 0:1],
        ).then_inc(s_stats, 1)  # in-order retire covers mpp's column too

        # ---- PE: partition-axis reduction of the 5 stats columns ------
        # s_stats counts three order-independent producers:
        # ones (lhsT), DVE row-sum columns, ACT S column.
        nc.tensor.matmul(
            acc[0:1, 0:5], ones[:], stats[:, 0:5], start=True, stop=True
        )._wait_ge(s_stats, 3).then_inc(s_pe, 1)

        # ---- DVE: totals to SBUF, denominator, scalar epilogue --------
        # Every same-engine RAW rides an s_v edge (see module docstring).
        nc.vector.tensor_copy(rw[0:1, 0:5], acc[0:1, 0:5])._wait_ge(
            s_pe, 1
        ).then_inc(s_v, 1)  # 3
        # d0 = L - n1
        nc.vector.tensor_scalar(
            out=c(16), in0=n1, scalar1=-1.0, scalar2=float(L),
            op0=_Alu.mult, op1=_Alu.add,
        )._wait_ge(s_v, 3).then_inc(s_v, 1)  # 4
        # denom = max(n1*d0, 1)
        nc.vector.scalar_tensor_tensor(
            out=c(17), in0=n1, scalar=c(16), in1=one_c,
            op0=_Alu.mult, op1=_Alu.max,
        )._wait_ge(s_v, 4).then_inc(s_v, 1)  # 5
        # r = 1/denom
        nc.vector.reciprocal(c(8), c(17))._wait_ge(s_v, 5)
        # a1 = 2S + T
        nc.vector.scalar_tensor_tensor(
            out=c(0), in0=S_, scalar=2.0, in1=T_, op0=_Alu.mult, op1=_Alu.add
        ).then_inc(s_v, 1)  # 6
        # w = s1 - S
        nc.vector.scalar_tensor_tensor(
            out=c(1), in0=S_, scalar=-1.0, in1=s1, op0=_Alu.mult, op1=_Alu.add
        ).then_inc(s_v, 1)  # 7
        # b1 = s2 - 2 s1
        nc.vector.scalar_tensor_tensor(
            out=c(2), in0=s1, scalar=-2.0, in1=s2, op0=_Alu.mult, op1=_Alu.add
        )
        # alpha = a1 - 2 s2
        nc.vector.scalar_tensor_tensor(
            out=c(3), in0=s2, scalar=-2.0, in1=c(0), op0=_Alu.mult, op1=_Alu.add
        )._wait_ge(s_v, 6)
        # q1 = (2 s1) * w
        nc.vector.scalar_tensor_tensor(
            out=c(4), in0=s1, scalar=2.0, in1=c(1), op0=_Alu.mult, op1=_Alu.mult
        )._wait_ge(s_v, 7).then_inc(s_v, 1)  # 8
        # q2 = L*b1 + q1
        nc.vector.scalar_tensor_tensor(
            out=c(5), in0=c(2), scalar=float(L), in1=c(4),
            op0=_Alu.mult, op1=_Alu.add,
        )._wait_ge(s_v, 8).then_inc(s_v, 1)  # 9: b1 covered by in-order retire
        # num = n1*alpha + q2
        nc.vector.scalar_tensor_tensor(
            out=c(6), in0=n1, scalar=c(3), in1=c(5), op0=_Alu.mult, op1=_Alu.add
        )._wait_ge(s_v, 9).then_inc(s_v, 1)  # 10: alpha covered likewise
        # out = num*r + 1
        nc.vector.scalar_tensor_tensor(
            out=c(9), in0=c(6), scalar=c(8), in1=one_c,
            op0=_Alu.mult, op1=_Alu.add,
        )._wait_ge(s_v, 10).then_inc(s_v, 1)  # 11: result ready for SP

        # ---- Pool: self-cleaning tail (no barriers) -------------------
        # s_done (the SP store) transitively implies every semaphore
        # reached its final value and all engines retired their last real
        # instruction. The wait rides ON the drain instruction itself.
        sems = (s_in, s_act, s_stats, s_pe, s_v, s_done)
        sem_lo = min(s.num for s in sems)
        sem_hi = max(s.num for s in sems)
        nc.gpsimd.dma_reset(range(sem_lo, sem_hi + 1))._wait_ge(s_done, 1)
        nc.gpsimd.sem_clear(range(sem_lo, sem_hi + 1))

    nc.compile()
    return nc


def _pack(pred_Y, true_Y):
    xin = np.empty((P, 2 * F), dtype=np.float32)
    xin[:, 0:F] = np.ascontiguousarray(pred_Y, dtype=np.float32).reshape(P, F)
    xin[:, F : 2 * F] = (
        np.ascontiguousarray(true_Y, dtype=np.int32).reshape(P, F).view(np.float32)
    )
    return xin


def _run(pred_Y, true_Y, **hw_kwargs):
    global _built
    if _built is None:
        _built = _build()
    in_map = {"xin": _pack(pred_Y, true_Y)}
    res = run_bass_kernel_spmd(
        _built, [in_map] * N_CORES, list(range(N_CORES)), **hw_kwargs
    )
    out = np.asarray(res.results[0]["out"], dtype=np.float32).reshape(())
    return out, res


def kernel(pred_Y, true_Y):
    out, _ = _run(pred_Y, true_Y)
    return out


# revision 19
# speedup vs baseline: 1.3373x; 1.0211x over previous
"""Trainium2 Bass kernel for nn_DIYloss_1709396984424.

Loss: for binary labels, mean over (one, zero) pairs of (1 + p[l] - p[k])^2
where p = sigmoid(pred_Y). The L^2 pairwise sum collapses to O(L) masked
reductions. With n1 = sum(m), s1 = sum(m*p), s2 = sum(m*p^2), S = sum(p),
T = sum(p^2):

    num   = n1*(T + 2S - 2*s2) + L*(s2 - 2*s1) + 2*s1*(s1 - S)
    denom = max(n1*(L - n1), 1)
    loss  = pair_sum/denom = 1 + num/denom

Each of the 8 cores receives the full (replicated) input and computes the
scalar on-device; core 0's output is returned.

Schedule (engines in parallel, no barriers anywhere):
  SP  : input DMA as the very first instruction, final 4-byte reg store.
  ACT : act-table load (auto), sigmoid, then a Copy-with-accum giving the
        S row sums (copy and sigmoid share one act table set).
  DVE : ones memset, mask cast + n1 row sums (hidden in the wait for the
        sigmoid), three fused product+row-sum ops, PSUM copy, denominator
        chain, scalar epilogue.
  PE  : one [128,5]x[128,1] matmul reducing the partition axis.
  Pool: const-0.0 memset (kept from the framework preamble; the sigmoid
        bias reads it microseconds later), end-of-program DMA-queue +
        semaphore reset (re-execution safety).

Hazard rules (learned the hard way): engine pipelines do NOT interlock
same-engine read-after-write on small operands — an instruction can read
an SBUF cell before the previous instruction's write retires. EVERY RAW
dependency therefore carries a semaphore edge; same-engine edges use the
counting semaphore s_v, where a wait on a later inc covers all earlier
instructions via in-order retirement. Each instruction has at most one
wait (the hardware limit).

The framework's startup barrier and three of its four const-tensor
memsets are suppressed at module-build time (the sigmoid bias uses the
kept const-0.0; nothing references the other three). NEFF executions are
serialized by completion, so the barrier only cost latency. This moves
the DMA trigger from t=666ns to t=0.
"""

import numpy as np

try:
    import concourse.bass as cbass  # noqa: F401
except ImportError:  # pragma: no cover - grading env should have it on path
    import sys

    sys.path.insert(0, "/opt/trn_rl_repo")
    import concourse.bass as cbass  # noqa: F401

from concourse import bacc, mybir
from concourse.bass_utils import run_bass_kernel_spmd

L = 8192
P = 128
F = L // P  # 64
N_CORES = 8

_f32 = mybir.dt.float32
_i32 = mybir.dt.int32
_Alu = mybir.AluOpType
_Act = mybir.ActivationFunctionType
_X = mybir.AxisListType.X

_built = None

# Suppress the framework init preamble (startup all-engine barrier and the
# const memsets other than f32-0.0, which the sigmoid bias uses) while
# constructing the module. The flag is only on during Bacc.__init__.
_suppress = {"on": False}
_orig_memset = cbass.BassSharedVectorInterface.memset
_orig_aeb = cbass.Bass.all_engine_barrier


def _memset_patched(self, ap, constant):
    if _suppress["on"] and constant != 0.0:
        return None
    return _orig_memset(self, ap, constant)


def _aeb_patched(self, *a, **k):
    if _suppress["on"]:
        return None
    return _orig_aeb(self, *a, **k)


cbass.BassSharedVectorInterface.memset = _memset_patched
cbass.Bass.all_engine_barrier = _aeb_patched


def _build():
    _suppress["on"] = True
    try:
        nc = bacc.Bacc(
            "TRN2", debug=False, target_bir_lowering=False, num_devices=N_CORES
        )
    finally:
        _suppress["on"] = False

    # cols 0:F = pred_Y (f32), cols F:2F = true_Y (int32 bitcast to f32)
    xin_d = nc.dram_tensor("xin", [P, 2 * F], _f32, kind="ExternalInput")
    out_d = nc.dram_tensor("out", [1, 1], _f32, kind="ExternalOutput")

    with (
        nc.sbuf_tensor("xt", [P, 2 * F], _f32) as xt,
        nc.sbuf_tensor("p", [P, F], _f32) as p,
        nc.sbuf_tensor("m1", [P, F], _f32) as m1,
        nc.sbuf_tensor("mp", [P, F], _f32) as mp,
        nc.sbuf_tensor("mpp", [P, F], _f32) as mpp,
        nc.sbuf_tensor("p2", [P, F], _f32) as p2,
        nc.sbuf_tensor("sc", [P, F], _f32) as sc,
        nc.sbuf_tensor("stats", [P, 8], _f32) as stats,
        nc.sbuf_tensor("ones", [P, 1], _f32) as ones,
        nc.sbuf_tensor("rw", [1, 32], _f32) as rw,
        nc.psum_tensor("acc", [1, 8], _f32) as acc,
        nc.semaphore("s_in") as s_in,
        nc.semaphore("s_act") as s_act,
        nc.semaphore("s_stats") as s_stats,
        nc.semaphore("s_pe") as s_pe,
        nc.semaphore("s_v") as s_v,
        nc.semaphore("s_done") as s_done,
    ):
        pred_v = xt[:, 0:F]
        true_v = xt[:, F : 2 * F].bitcast(_i32)

        # totals after the PSUM copy: rw[0,0:5] = [T, n1, s1, s2, S]
        T_ = rw[0:1, 0:1]
        n1 = rw[0:1, 1:2]
        s1 = rw[0:1, 2:3]
        s2 = rw[0:1, 3:4]
        S_ = rw[0:1, 4:5]

        def c(i):  # epilogue scratch cells
            return rw[0:1, 8 + i : 9 + i]

        one_c = ones[0:1, 0:1]

        # ---- SP: input DMA first, result store last -------------------
        nc.sync.dma_start(xt[:], xin_d[:]).then_inc(s_in, 16)
        reg = nc.sync.alloc_register()
        nc.sync.reg_load(reg, c(9).bitcast(_i32))._wait_ge(s_v, 11)
        nc.sync.store(out_d[0:1, 0:1].bitcast(_i32), reg).then_inc(s_done, 1)

        # ---- ACT: sigmoid, then Copy-with-accum for S row sums --------
        # bias 0.0 resolves to the kept const-0.0 tile (Pool writes it at
        # t~100ns; the earliest possible sigmoid start is ~2.3us later).
        nc.scalar.activation(p[:], pred_v, _Act.Sigmoid)._wait_ge(
            s_in, 16
        ).then_inc(s_act, 1)
        nc.scalar.activation(
            sc[:], p[:], _Act.Copy, accum_out=stats[:, 4:5]
        )._wait_ge(s_act, 1).then_inc(s_stats, 1)  # own-engine RAW on p

        # ---- DVE: ones, mask cast + n1, fused product sums ------------
        nc.vector.memset(ones[:], 1.0).then_inc(s_stats, 1)
        nc.vector.tensor_copy(m1[:], true_v)._wait_ge(s_in, 16).then_inc(
            s_v, 1
        )  # int -> f32 cast (values 0/1)
        # n1 row sums; hides in the wait for the sigmoid result
        nc.vector.tensor_reduce(
            stats[:, 1:2], m1[:], axis=_X, op=_Alu.add
        )._wait_ge(s_v, 1)
        # stats cols: 0 = T partials, 1 = n1, 2 = s1, 3 = s2, 4 = S
        # Order mp, p2, mpp: p2 depends only on p, so it executes while the
        # mp->mpp same-engine RAW edge (~95ns) resolves.
        nc.vector.scalar_tensor_tensor(
            out=mp[:], in0=m1[:], scalar=1.0, in1=p[:],
            op0=_Alu.mult, op1=_Alu.mult, accum_out=stats[:, 2:3],
        )._wait_ge(s_act, 1).then_inc(s_v, 1)  # 2
        nc.vector.scalar_tensor_tensor(
            out=p2[:], in0=p[:], scalar=1.0, in1=p[:],
            op0=_Alu.mult, op1=_Alu.mult, accum_out=stats[:, 0:1],
        )
        nc.vector.scalar_tensor_tensor(
            out=mpp[:], in0=mp[:], scalar=1.0, in1=p[:],
            op0=_Alu.mult, op1=_Alu.mult, accum_out=stats[:, 3:4],
        )._wait_ge(s_v, 2).then_inc(s_stats, 1)  # in-order retire covers p2

        # ---- PE: partition-axis reduction of the 5 stats columns ------
        # s_stats counts three order-independent producers:
        # ones (lhsT), DVE row-sum columns, ACT S column.
        nc.tensor.matmul(
            acc[0:1, 0:5], ones[:], stats[:, 0:5], start=True, stop=True
        )._wait_ge(s_stats, 3).then_inc(s_pe, 1)

        # ---- DVE: totals to SBUF, denominator, scalar epilogue --------
        # Every same-engine RAW rides an s_v edge (see module docstring).
        nc.vector.tensor_copy(rw[0:1, 0:5], acc[0:1, 0:5])._wait_ge(
            s_pe, 1
        ).then_inc(s_v, 1)  # 3
        # d0 = L - n1
        nc.vector.tensor_scalar(
            out=c(16), in0=n1, scalar1=-1.0, scalar2=float(L),
            op0=_Alu.mult, op1=_Alu.add,
        )._wait_ge(s_v, 3).then_inc(s_v, 1)  # 4
        # denom = max(n1*d0, 1)
        nc.vector.scalar_tensor_tensor(
            out=c(17), in0=n1, scalar=c(16), in1=one_c,
            op0=_Alu.mult, op1=_Alu.max,
        )._wait_ge(s_v, 4).then_inc(s_v, 1)  # 5
        # r = 1/denom
        nc.vector.reciprocal(c(8), c(17))._wait_ge(s_v, 5)
        # a1 = 2S + T
        nc.vector.scalar_tensor_tensor(
            out=c(0), in0=S_, scalar=2.0, in1=T_, op0=_Alu.mult, op1=_Alu.add
        ).then_inc(s_v, 1)  # 6
        # w = s1 - S
        nc.vector.scalar_tensor_tensor(
            out=c(1), in0=S_, scalar=-1.0, in1=s1, op0=_Alu.mult, op1=_Alu.add
        ).then_inc(s_v, 1)  # 7
        # b1 = s2 - 2 s1
        nc.vector.scalar_tensor_tensor(
            out=c(2), in0=s1, scalar=-2.0, in1=s2, op0=_Alu.mult, op1=_Alu.add
        )
        # alpha = a1 - 2 s2
        nc.vector.scalar_tensor_tensor(
            out=c(3), in0=s2, scalar=-2.0, in1=c(0), op0=_Alu.mult, op1=_Alu.add
        )._wait_ge(s_v, 6)
        # q1 = (2 s1) * w
        nc.vector.scalar_tensor_tensor(
            out=c(4), in0=s1, scalar=2.0, in1=c(1), op0=_Alu.mult, op1=_Alu.mult
        )._wait_ge(s_v, 7).then_inc(s_v, 1)  # 8
        # q2 = L*b1 + q1
        nc.vector.scalar_tensor_tensor(
            out=c(5), in0=c(2), scalar=float(L), in1=c(4),
            op0=_Alu.mult, op1=_Alu.add,
        )._wait_ge(s_v, 8).then_inc(s_v, 1)  # 9: b1 covered by in-order retire
        # num = n1*alpha + q2
        nc.vector.scalar_tensor_tensor(
            out=c(6), in0=n1, scalar=c(3), in1=c(5), op0=_Alu.mult, op1=_Alu.add
        )._wait_ge(s_v, 9).then_inc(s_v, 1)  # 10: alpha covered likewise
        # out = num*r + 1
        nc.vector.scalar_tensor_tensor(
            out=c(9), in0=c(6), scalar=c(8), in1=one_c,
            op0=_Alu.mult, op1=_Alu.add,
        )._wait_ge(s_v, 10).then_inc(s_v, 1)  # 11: result ready for SP

        # ---- Pool: self-cleaning tail (no barriers) -------------------
        # s_done (the SP store) transitively implies every semaphore
        # reached its final value and all engines retired their last real
        # instruction. The wait rides ON the drain instruction itself.
        sems = (s_in, s_act, s_stats, s_pe, s_v, s_done)
        sem_lo = min(s.num for s in sems)
        sem_hi = max(s.num for s in sems)
        nc.gpsimd.dma_reset(range(sem_lo, sem_hi + 1))._wait_ge(s_done, 1)
        nc.gpsimd.sem_clear(range(sem_lo, sem_hi + 1))

    nc.compile()
    return nc


def _pack(pred_Y, true_Y):
    xin = np.empty((P, 2 * F), dtype=np.float32)
    xin[:, 0:F] = np.ascontiguousarray(pred_Y, dtype=np.float32).reshape(P, F)
    xin[:, F : 2 * F] = (
        np.ascontiguousarray(true_Y, dtype=np.int32).reshape(P, F).view(np.float32)
    )
    return xin


def _run(pred_Y, true_Y, **hw_kwargs):
    global _built
    if _built is None:
        _built = _build()
    in_map = {"xin": _pack(pred_Y, true_Y)}
    res = run_bass_kernel_spmd(
        _built, [in_map] * N_CORES, list(range(N_CORES)), **hw_kwargs
    )
    out = np.asarray(res.results[0]["out"], dtype=np.float32).reshape(())
    return out, res


def kernel(pred_Y, true_Y):
    out, _ = _run(pred_Y, true_Y)
    return out
